# revision 2
# baseline (speedup 1.0000x reference)
"""MoE network TRN2 kernel: feature-parallel sharding, minimal per-core input bytes.

Collectives are restricted to AllReduce/AllGather only: ReduceScatter in this
runtime triggers a pathological ~100x whole-NEFF slowdown (measured), so
layer 2 is output-feature-sharded and consumes AllGathered activations
instead of combining contraction partials with a ReduceScatter.

Sharding: core c owns output-feature slice j in [256c, 256c+256) of layer-1
experts (exp1_W[:, :, slice], 8 MB fp32r) and the matching input-row slice of
layer-2 experts (exp2_W[:, slice, :], 4 MB fp16). Tokens are sharded only for
the initial BatchNorm stats + gate-1 (512 tokens/core); an AllGather of the
normalized activations gives every core all 4096 tokens, so layer-1 output
z1[:, slice] is complete locally and BN2 statistics are exact with no
cross-core reduce. For layer 2 the normalized activations are re-gathered in
8 token chunks (AllGathers that pipeline ahead of compute) and each core
computes its 128-wide output-feature slice of z2 completely, so only a
partial head contraction leaves the core; partial outputs are summed on the
host. Per-core input bytes: ~14.3 MB (vs 153 MB for replicated weights).
"""
import os
import sys

import numpy as np

sys.path.insert(0, "/opt/trn_rl_repo")

B, DIN, DHID, DH2, E = 4096, 1024, 2048, 1024, 8
NCORES = 8
BL = B // NCORES            # 512 tokens/core for stats+gate1
IC1 = DIN // 128            # 8 input chunks
JS1 = DHID // NCORES        # 256 = layer-1 output slice per core
JC1L = JS1 // 128           # 2 local j chunks
IC2L = JC1L                 # 2 local input chunks for layer 2
JC2 = DH2 // 128            # 8 output chunks, layer 2
NTC = 8                     # token chunks of 512
EPS = 1e-5

_CACHE = {}


def _round_fp32r(x):
    b = np.ascontiguousarray(x, np.float32).view(np.uint32).astype(np.uint64)
    half = np.uint64(1 << 11)
    one = np.uint64(1)
    lsb = (b >> np.uint64(12)) & one
    b = (b + half - one + lsb) & ~np.uint64((1 << 12) - 1)
    return (b & np.uint64(0xFFFFFFFF)).astype(np.uint32).view(np.float32)


def _build():
    import concourse.bass_isa as bass_isa
    import concourse.mybir as mybir
    import concourse.tile as tile
    from concourse import bacc

    f32 = mybir.dt.float32
    f32r = mybir.dt.float32r
    f16 = mybir.dt.float16
    AF = mybir.ActivationFunctionType
    OP = mybir.AluOpType
    RG = [list(range(NCORES))]

    nc = bacc.Bacc(None, target_bir_lowering=False, num_devices=NCORES)
    mask = 0 if os.environ.get("KERNEL_NOCC") else int(os.environ.get("KERNEL_CC_MASK", "15"))
    nocc = mask != 15  # any stubbed collective forces non-Shared buffers
    cc_bn1 = bool(mask & 1)
    cc_ag = bool(mask & 2)
    cc_lg2 = bool(mask & 4)
    cc_rs = bool(mask & 8)

    # ---- per-core inputs (the io-byte budget)
    xt = nc.dram_tensor("xt", [IC1, 128, BL], f32, kind="ExternalInput")
    w1 = nc.dram_tensor("w1", [E, IC1, 128, JS1], f32r, kind="ExternalInput")
    w2 = nc.dram_tensor("w2", [E, DHID // 128, 128, 128], f16,
                        kind="ExternalInput")
    b1 = nc.dram_tensor("b1", [E, JC1L, 128], f32, kind="ExternalInput")
    b2 = nc.dram_tensor("b2", [E, 128], f32, kind="ExternalInput")
    g1w = nc.dram_tensor("g1w", [IC1, 128, E], f32, kind="ExternalInput")
    g2w = nc.dram_tensor("g2w", [IC2L, 128, E], f32, kind="ExternalInput")
    g1b = nc.dram_tensor("g1b", [E, 1], f32, kind="ExternalInput")
    g2b = nc.dram_tensor("g2b", [E, 1], f32, kind="ExternalInput")
    bn1g = nc.dram_tensor("bn1g", [IC1, 128], f32, kind="ExternalInput")
    bn1b = nc.dram_tensor("bn1b", [IC1, 128], f32, kind="ExternalInput")
    bn2g = nc.dram_tensor("bn2g", [JC1L, 128], f32, kind="ExternalInput")
    bn2b = nc.dram_tensor("bn2b", [JC1L, 128], f32, kind="ExternalInput")
    ow = nc.dram_tensor("ow", [128, 1], f32, kind="ExternalInput")
    obo = nc.dram_tensor("obo", [1, 1], f32, kind="ExternalInput")
    out = nc.dram_tensor("out", [B, 1], f32, kind="ExternalOutput")

    with tile.TileContext(nc) as tc:
        with tc.tile_pool(name="const", bufs=1) as const, \
             tc.tile_pool(name="small", bufs=1) as small, \
             tc.tile_pool(name="gsc", bufs=12) as gsc, \
             tc.tile_pool(name="gtmp", bufs=3) as gtmp, \
             tc.tile_pool(name="dram", bufs=1, space="DRAM") as dram:

            # ---- small parameter loads
            bn1g_t = const.tile([128, IC1], f32)
            bn1b_t = const.tile([128, IC1], f32)
            bn2g_t = const.tile([128, JC1L], f32)
            bn2b_t = const.tile([128, JC1L], f32)
            nc.sync.dma_start(out=bn1g_t[:], in_=bn1g.rearrange("c p -> p c"))
            nc.sync.dma_start(out=bn1b_t[:], in_=bn1b.rearrange("c p -> p c"))
            nc.sync.dma_start(out=bn2g_t[:], in_=bn2g.rearrange("c p -> p c"))
            nc.sync.dma_start(out=bn2b_t[:], in_=bn2b.rearrange("c p -> p c"))
            g1w_t = const.tile([128, IC1, E], f32)
            g2w_t = const.tile([128, IC2L, E], f32)
            nc.sync.dma_start(out=g1w_t[:], in_=g1w.rearrange("c p e -> p c e"))
            nc.sync.dma_start(out=g2w_t[:], in_=g2w.rearrange("c p e -> p c e"))
            g1b_t = const.tile([E, 1], f32)
            g2b_t = const.tile([E, 1], f32)
            nc.sync.dma_start(out=g1b_t[:], in_=g1b[:])
            nc.sync.dma_start(out=g2b_t[:], in_=g2b[:])
            b1_tf = small.tile([E, JC1L, 128], f32, name="b1_tf")
            nc.sync.dma_start(out=b1_tf[:], in_=b1[:])
            b1r = const.tile([E, JC1L, 128], f32r)
            nc.vector.tensor_copy(b1r[:], b1_tf[:])
            b2_tf = small.tile([E, 128], f32, name="b2_tf")
            nc.sync.dma_start(out=b2_tf[:], in_=b2[:])
            b2r = const.tile([E, 128], f32r)
            nc.vector.tensor_copy(b2r[:], b2_tf[:])
            ow_t = const.tile([128, 1], f32)
            nc.sync.dma_start(out=ow_t[:], in_=ow[:])
            obo_t = const.tile([128, 1], f32)
            nc.sync.dma_start(
                out=obo_t[:],
                in_=obo[0:1, 0:1].partition_broadcast(128).squeeze(1))

            junk = const.tile([128, 512], f32)

            # ---- helpers
            def bn_finish(s1, s2, icn, gamma_t, beta_t, name):
                mu = small.tile([128, icn], f32, name=f"mu_{name}")
                ex2 = small.tile([128, icn], f32, name=f"ex2_{name}")
                nc.vector.tensor_scalar(mu[:], s1[:], 1.0 / B, None, OP.mult)
                nc.vector.tensor_scalar(ex2[:], s2[:], 1.0 / B, None, OP.mult)
                var = small.tile([128, icn], f32, name=f"var_{name}")
                nc.vector.tensor_tensor(out=var[:], in0=mu[:], in1=mu[:], op=OP.mult)
                nc.vector.tensor_tensor(out=var[:], in0=ex2[:], in1=var[:],
                                        op=OP.subtract)
                vare = small.tile([128, icn], f32, name=f"vare_{name}")
                nc.vector.tensor_scalar(vare[:], var[:], EPS, None, OP.add)
                sd = small.tile([128, icn], f32, name=f"sd_{name}")
                nc.scalar.activation(sd[:], vare[:], AF.Sqrt)
                rstd = small.tile([128, icn], f32, name=f"rstd_{name}")
                nc.vector.reciprocal(rstd[:], sd[:])
                sv = small.tile([128, icn], f32, name=f"sv_{name}")
                bv = small.tile([128, icn], f32, name=f"bv_{name}")
                nc.vector.tensor_tensor(out=sv[:], in0=rstd[:], in1=gamma_t[:],
                                        op=OP.mult)
                nc.vector.tensor_tensor(out=bv[:], in0=mu[:], in1=sv[:], op=OP.mult)
                nc.vector.tensor_tensor(out=bv[:], in0=beta_t[:], in1=bv[:],
                                        op=OP.subtract)
                return sv, bv

            def gating_chain(lg, width, name):
                """top-2 masked softmax over E=8 partitions; lg [E, width] f32.
                Returns gat tile [E, width] f32 from the gtmp pool."""
                m1 = gsc.tile([E, width], f32, tag="g", name=f"m1_{name}")
                nc.gpsimd.partition_all_reduce(m1[:], lg[:], channels=E,
                                               reduce_op=bass_isa.ReduceOp.max)
                ismax = gsc.tile([E, width], f32, tag="g", name=f"ismax_{name}")
                nc.vector.tensor_tensor(out=ismax[:], in0=lg[:], in1=m1[:],
                                        op=OP.is_equal)
                cnt = gsc.tile([E, width], f32, tag="g", name=f"cnt_{name}")
                nc.gpsimd.partition_all_reduce(cnt[:], ismax[:], channels=E,
                                               reduce_op=bass_isa.ReduceOp.add)
                tmp = gsc.tile([E, width], f32, tag="g", name=f"tmp_{name}")
                nc.vector.scalar_tensor_tensor(
                    out=tmp[:], in0=ismax[:], scalar=-1e30, in1=lg[:],
                    op0=OP.mult, op1=OP.add)
                m2 = gsc.tile([E, width], f32, tag="g", name=f"m2_{name}")
                nc.gpsimd.partition_all_reduce(m2[:], tmp[:], channels=E,
                                               reduce_op=bass_isa.ReduceOp.max)
                c2m = gsc.tile([E, width], f32, tag="g", name=f"c2m_{name}")
                nc.vector.tensor_scalar(c2m[:], cnt[:], 1.5, None, OP.is_ge)
                dif = gsc.tile([E, width], f32, tag="g", name=f"dif_{name}")
                nc.vector.tensor_tensor(out=dif[:], in0=m1[:], in1=m2[:],
                                        op=OP.subtract)
                nc.vector.tensor_tensor(out=dif[:], in0=dif[:], in1=c2m[:],
                                        op=OP.mult)
                v2 = gsc.tile([E, width], f32, tag="g", name=f"v2_{name}")
                nc.vector.tensor_tensor(out=v2[:], in0=dif[:], in1=m2[:], op=OP.add)
                msk = gsc.tile([E, width], f32, tag="g", name=f"msk_{name}")
                nc.vector.tensor_tensor(out=msk[:], in0=lg[:], in1=v2[:],
                                        op=OP.is_ge)
                d = gsc.tile([E, width], f32, tag="g", name=f"d_{name}")
                nc.vector.tensor_tensor(out=d[:], in0=lg[:], in1=m1[:],
                                        op=OP.subtract)
                exd = gsc.tile([E, width], f32, tag="g", name=f"exd_{name}")
                nc.scalar.activation(exd[:], d[:], AF.Exp)
                exm = gsc.tile([E, width], f32, tag="g", name=f"exm_{name}")
                nc.vector.tensor_tensor(out=exm[:], in0=exd[:], in1=msk[:],
                                        op=OP.mult)
                den = gsc.tile([E, width], f32, tag="g", name=f"den_{name}")
                nc.gpsimd.partition_all_reduce(den[:], exm[:], channels=E,
                                               reduce_op=bass_isa.ReduceOp.add)
                rden = gsc.tile([E, width], f32, tag="g", name=f"rden_{name}")
                nc.vector.reciprocal(rden[:], den[:])
                gat = gtmp.tile([E, width], f32, tag="gat", name=f"gat_{name}")
                nc.vector.tensor_tensor(out=gat[:], in0=exm[:], in1=rden[:],
                                        op=OP.mult)
                return gat

            # gates for all tokens as f32r (bias-matmul rhs); persistent
            g1ar = small.tile([E, B], f32r, name="g1ar")
            g2ar = small.tile([E, B], f32r, name="g2ar")
            ysb = small.tile([128, 4 * NTC], f32, name="ysb")

            # collective DRAM buffers
            bn1_pl = dram.tile([128, 2 * IC1], f32, name="bn1_pl")
            _shr = "Local"
            bn1_ps = dram.tile([128, 2 * IC1], f32, addr_space=_shr,
                               name="bn1_ps")
            ag_xi = dram.tile([128, IC1, BL], f32, name="ag_xi")
            ag_xo = dram.tile([NCORES, 128, IC1, BL], f32, addr_space=_shr,
                              name="ag_xo")
            ag_gi = dram.tile([E, BL], f32, name="ag_gi")
            ag_go = dram.tile([NCORES, E, BL], f32, addr_space=_shr,
                              name="ag_go")
            gd2 = dram.tile([E, B], f16, name="gd2")
            ag2_is = [dram.tile([128, IC2L, 512], f16, name=f"ag2i_{t8}")
                      for t8 in range(NTC)]
            ag2_os = [dram.tile([NCORES, 128, IC2L, 512], f16,
                                name=f"ag2o_{t8}") for t8 in range(NTC)]
            lg2_pls = [dram.tile([E, 512], f32, name=f"lg2pl_{t8}")
                       for t8 in range(NTC)]
            lg2_pss = [dram.tile([E, 512], f32, name=f"lg2ps_{t8}")
                       for t8 in range(NTC)]

            # ================= phase 0: BN1 stats (AllReduce) + gate1 =========
            with tc.tile_pool(name="p0", bufs=1) as p0:
                xtf = p0.tile([128, IC1, BL], f32, name="xtf")
                nc.sync.dma_start(out=xtf[:], in_=xt.rearrange("c p t -> p c t"))
                s1x = small.tile([128, IC1], f32, name="s1x")
                s2x = small.tile([128, IC1], f32, name="s2x")
                for ic in range(IC1):
                    nc.vector.tensor_reduce(s1x[:, ic:ic + 1], xtf[:, ic, :],
                                            mybir.AxisListType.X, OP.add)
                    nc.scalar.activation(junk[:], xtf[:, ic, :], AF.Square,
                                         accum_out=s2x[:, ic:ic + 1])
                pk = small.tile([128, 2 * IC1], f32, name="pk_bn1")
                nc.vector.tensor_copy(pk[:, :IC1], s1x[:])
                nc.vector.tensor_copy(pk[:, IC1:], s2x[:])
                nc.sync.dma_start(out=bn1_pl[:], in_=pk[:])
                if not cc_bn1:
                    nc.sync.dma_start(out=bn1_ps[:], in_=bn1_pl[:])
                else:
                    nc.gpsimd.collective_compute(
                        "AllReduce", OP.add, replica_groups=RG,
                        ins=[bn1_pl[:]], outs=[bn1_ps[:]])
                gl = small.tile([128, 2 * IC1], f32, name="gl_bn1")
                nc.sync.dma_start(out=gl[:], in_=bn1_ps[:])
                sv1, bv1 = bn_finish(gl[:, :IC1], gl[:, IC1:], IC1,
                                     bn1g_t, bn1b_t, "bn1")

                xnc = p0.tile([128, IC1, BL], f32, name="xnc")
                for ic in range(IC1):
                    nc.vector.tensor_scalar(xnc[:, ic, :], xtf[:, ic, :],
                                            sv1[:, ic:ic + 1], bv1[:, ic:ic + 1],
                                            OP.mult, OP.add)

                # gate1 on local tokens (exact fp32)
                with tc.tile_pool(name="psg1", bufs=1, space="PSUM") as psg1:
                    psg = psg1.tile([E, BL], f32)
                    for ic in range(IC1):
                        nc.tensor.matmul(psg[:], lhsT=g1w_t[:, ic, :],
                                         rhs=xnc[:, ic, :],
                                         start=(ic == 0), stop=(ic == IC1 - 1))
                    lg1 = gsc.tile([E, BL], f32, tag="g", name="lg1")
                    nc.vector.tensor_scalar(lg1[:], psg[:], g1b_t[:], None, OP.add)
                gat1 = gating_chain(lg1, BL, "g1")

                # AllGather normalized activations + gates
                nc.sync.dma_start(out=ag_xi[:], in_=xnc[:])
                if not cc_ag:
                    for _r in range(NCORES):
                        nc.sync.dma_start(out=ag_xo[_r], in_=ag_xi[:])
                else:
                    nc.gpsimd.collective_compute(
                        "AllGather", OP.bypass, replica_groups=RG,
                        ins=[ag_xi[:]], outs=[ag_xo[:]])
                nc.sync.dma_start(out=ag_gi[:], in_=gat1[:])
                if not cc_ag:
                    for _r in range(NCORES):
                        nc.sync.dma_start(out=ag_go[_r], in_=ag_gi[:])
                else:
                    nc.gpsimd.collective_compute(
                        "AllGather", OP.bypass, replica_groups=RG,
                        ins=[ag_gi[:]], outs=[ag_go[:]])

            # unpack gates for all tokens -> f32r
            for r in range(NCORES):
                gtr = gtmp.tile([E, BL], f32, tag="gat", name=f"g1u_{r}")
                nc.sync.dma_start(out=gtr[:], in_=ag_go[r])
                nc.vector.tensor_copy(g1ar[:, r * BL:(r + 1) * BL], gtr[:])

            s1z = small.tile([128, JC1L, NTC], f32, name="s1z")
            s2z = small.tile([128, JC1L, NTC], f32, name="s2z")

            with tc.tile_pool(name="xn2p", bufs=1) as xn2p:
                xn2 = xn2p.tile([128, IC2L, B], f32)

                # ============= layer 1: feature-sliced, all tokens ============
                with tc.tile_pool(name="z1p", bufs=1) as z1p:
                    z1sb = z1p.tile([128, JC1L, B], f32)
                    with tc.tile_pool(name="xqp", bufs=2) as xqp, \
                         tc.tile_pool(name="htp", bufs=3) as htp, \
                         tc.tile_pool(name="gb1p", bufs=2) as gb1p, \
                         tc.tile_pool(name="w1p", bufs=6) as w1p, \
                         tc.tile_pool(name="ps1", bufs=8, space="PSUM") as ps1:
                        for t8 in range(NTC):
                            sl = slice(t8 * 512, (t8 + 1) * 512)
                            xq = xqp.tile([128, IC1, 512], f32, tag="xq",
                                          name=f"xq_{t8}")
                            nc.sync.dma_start(out=xq[:], in_=ag_xo[t8])
                            psq = [ps1.tile([128, 512], f32, tag="ps",
                                            name=f"ps1_{t8}_{jc}")
                                   for jc in range(JC1L)]
                            for e in range(E):
                                gb = gb1p.tile([128, 512], f32, tag="gb",
                                               name=f"gb_{t8}_{e}")
                                nc.sync.dma_start(
                                    out=gb[:],
                                    in_=ag_go[t8, e:e + 1, :]
                                    .partition_broadcast(128).squeeze(1))
                                for ic in range(IC1):
                                    w1t = w1p.tile([128, JS1], f32r, tag="ws",
                                                   name=f"w1_{t8}_{e}_{ic}")
                                    nc.sync.dma_start(out=w1t[:], in_=w1[e, ic])
                                    ht = htp.tile([128, 512], f32r, tag="ht",
                                                  name=f"ht_{t8}_{e}_{ic}")
                                    nc.vector.tensor_tensor(
                                        out=ht[:], in0=xq[:, ic, :], in1=gb[:],
                                        op=OP.mult)
                                    for jc in range(JC1L):
                                        nc.tensor.matmul(
                                            psq[jc][:],
                                            lhsT=w1t[:, jc * 128:(jc + 1) * 128],
                                            rhs=ht[:],
                                            start=(e == 0 and ic == 0),
                                            stop=False)
                            for jc in range(JC1L):
                                nc.tensor.matmul(
                                    psq[jc][:], lhsT=b1r[:, jc, :],
                                    rhs=g1ar[:, sl], start=False, stop=True)
                                nc.vector.tensor_copy(z1sb[:, jc, sl], psq[jc][:])
                                nc.vector.tensor_reduce(
                                    s1z[:, jc, t8:t8 + 1], z1sb[:, jc, sl],
                                    mybir.AxisListType.X, OP.add)
                                nc.scalar.activation(
                                    junk[:], z1sb[:, jc, sl], AF.Square,
                                    accum_out=s2z[:, jc, t8:t8 + 1])

                    # ============= BN2 (local, exact) + relu ==================
                    s1f = small.tile([128, JC1L], f32, name="s1f")
                    s2f = small.tile([128, JC1L], f32, name="s2f")
                    for jc in range(JC1L):
                        nc.vector.tensor_reduce(s1f[:, jc:jc + 1], s1z[:, jc, :],
                                                mybir.AxisListType.X, OP.add)
                        nc.vector.tensor_reduce(s2f[:, jc:jc + 1], s2z[:, jc, :],
                                                mybir.AxisListType.X, OP.add)
                    sv2, bv2 = bn_finish(s1f, s2f, JC1L, bn2g_t, bn2b_t, "bn2")
                    for jc in range(JC1L):
                        nc.scalar.activation(xn2[:, jc, :], z1sb[:, jc, :],
                                             AF.Relu, bias=bv2[:, jc:jc + 1],
                                             scale=sv2[:, jc:jc + 1])

                # ==== gate2 + xn2 re-gather, pipelined per token chunk ========
                # per chunk: stage fp16 xn2 + AllGather it, compute partial
                # gate logits, tiny Mesh AllReduce, then the top-2 chain —
                # interleaved on the cc stream so layer 2 starts early
                with tc.tile_pool(name="psg2", bufs=2, space="PSUM") as psg2:
                    for t8 in range(NTC):
                        sl = slice(t8 * 512, (t8 + 1) * 512)
                        xf = gtmp.tile([128, IC2L, 512], f16, tag="xf",
                                       name=f"xf_{t8}", bufs=2)
                        nc.vector.tensor_copy(xf[:], xn2[:, :, sl])
                        nc.sync.dma_start(out=ag2_is[t8][:], in_=xf[:])
                        if not cc_rs:
                            for _r in range(NCORES):
                                nc.sync.dma_start(out=ag2_os[t8][_r],
                                                  in_=ag2_is[t8][:])
                        else:
                            nc.gpsimd.collective_compute(
                                "AllGather", OP.bypass, replica_groups=RG,
                                ins=[ag2_is[t8][:]], outs=[ag2_os[t8][:]])
                        psl = psg2.tile([E, 512], f32, tag="ps",
                                        name=f"pg2_{t8}")
                        for ic in range(IC2L):
                            nc.tensor.matmul(psl[:], lhsT=g2w_t[:, ic, :],
                                             rhs=xn2[:, ic, sl],
                                             start=(ic == 0),
                                             stop=(ic == IC2L - 1))
                        lgs = gtmp.tile([E, 512], f32, tag="gat",
                                        name=f"lgp_{t8}")
                        nc.vector.tensor_copy(lgs[:], psl[:])
                        nc.sync.dma_start(out=lg2_pls[t8][:], in_=lgs[:])
                        if not cc_lg2:
                            nc.sync.dma_start(out=lg2_pss[t8][:],
                                              in_=lg2_pls[t8][:])
                        else:
                            nc.gpsimd.collective_compute(
                                "AllReduce", OP.add, replica_groups=RG,
                                ins=[lg2_pls[t8][:]], outs=[lg2_pss[t8][:]])
                        lgc = gsc.tile([E, 512], f32, tag="g", name=f"lg2_{t8}")
                        nc.sync.dma_start(out=lgc[:], in_=lg2_pss[t8][:])
                        nc.vector.tensor_scalar(lgc[:], lgc[:], g2b_t[:], None,
                                                OP.add)
                        gat2 = gating_chain(lgc, 512, f"g2_{t8}")
                        nc.vector.tensor_copy(g2ar[:, sl], gat2[:])
                        g16 = gtmp.tile([E, 512], f16, tag="gat",
                                        name=f"g16_{t8}")
                        nc.vector.tensor_copy(g16[:], gat2[:])
                        nc.sync.dma_start(out=gd2[:, sl], in_=g16[:])

                with tc.tile_pool(name="w2p", bufs=1) as w2p, \
                     tc.tile_pool(name="xt2p", bufs=3) as xt2p, \
                     tc.tile_pool(name="ht2p", bufs=3) as ht2p, \
                     tc.tile_pool(name="gb2p", bufs=10) as gb2p, \
                     tc.tile_pool(name="z2p", bufs=2) as z2p, \
                     tc.tile_pool(name="ps2", bufs=4, space="PSUM") as ps2, \
                     tc.tile_pool(name="psh", bufs=2, space="PSUM") as psh:
                    w2sb = w2p.tile([128, E, DHID // 128, 128], f16)
                    nc.sync.dma_start(out=w2sb[:],
                                      in_=w2.rearrange("e i p f -> p e i f"))
                    for t8 in range(NTC):
                        sl = slice(t8 * 512, (t8 + 1) * 512)
                        gb2s = []
                        for e in range(E):
                            gb2 = gb2p.tile([128, 512], f16, tag="gb2",
                                            name=f"gb2_{t8}_{e}")
                            nc.sync.dma_start(
                                out=gb2[:],
                                in_=gd2[e:e + 1, sl].partition_broadcast(128)
                                .squeeze(1))
                            gb2s.append(gb2)
                        p2 = ps2.tile([128, 512], f32, tag="ps",
                                      name=f"p2_{t8}")
                        for r in range(NCORES):
                            for ic in range(IC2L):
                                xt2 = xt2p.tile([128, 512], f16, tag="xt2",
                                                name=f"xt2_{t8}_{r}_{ic}")
                                nc.sync.dma_start(out=xt2[:],
                                                  in_=ag2_os[t8][r, :, ic, :])
                                for e in range(E):
                                    ht2 = ht2p.tile([128, 512], f16, tag="ht2",
                                                    name=f"ht2_{t8}_{r}_{ic}_{e}")
                                    nc.vector.tensor_tensor(
                                        out=ht2[:], in0=xt2[:], in1=gb2s[e][:],
                                        op=OP.mult)
                                    nc.tensor.matmul(
                                        p2[:],
                                        lhsT=w2sb[:, e, r * IC2L + ic, :],
                                        rhs=ht2[:],
                                        start=(r == 0 and ic == 0 and e == 0),
                                        stop=False)
                        nc.tensor.matmul(p2[:], lhsT=b2r[:], rhs=g2ar[:, sl],
                                         start=False, stop=True)
                        z2a = z2p.tile([128, 512], f32, tag="z2",
                                       name=f"z2a_{t8}")
                        nc.scalar.activation(z2a[:], p2[:], AF.Relu)
                        for k in range(4):
                            ph = psh.tile([128, 1], f32, tag="ph",
                                          name=f"ph_{t8}_{k}")
                            nc.tensor.matmul(ph[:],
                                             lhsT=z2a[:, k * 128:(k + 1) * 128],
                                             rhs=ow_t[:], start=True, stop=True)
                            nc.vector.tensor_scalar(
                                ysb[:, t8 * 4 + k:t8 * 4 + k + 1],
                                ph[:], obo_t[:], None, OP.add)

            nc.sync.dma_start(out=out.rearrange("(c p) m -> p (c m)", p=128),
                              in_=ysb[:])

    nc.finalize()
    return nc


def _get_nc():
    if "nc" not in _CACHE:
        _CACHE["nc"] = _build()
    return _CACHE["nc"]


def kernel(x, bn1_gamma, bn1_beta, bn2_gamma, bn2_beta,
           gate1_W, gate1_b, exp1_W, exp1_b,
           gate2_W, gate2_b, exp2_W, exp2_b,
           out_W, out_b):
    from concourse.bass_utils import run_bass_kernel_spmd

    nc = _get_nc()

    xT = np.ascontiguousarray(np.asarray(x, np.float32).T)   # [DIN, B]
    w1f = np.asarray(exp1_W, np.float32)                     # [E, DIN, DHID]
    w2f = np.asarray(exp2_W, np.float32)                     # [E, DHID, DH2]
    b1f = np.asarray(exp1_b, np.float32)                     # [E, DHID]
    b2f = np.asarray(exp2_b, np.float32)                     # [E, DH2]
    common = {
        "g1w": np.asarray(gate1_W, np.float32).reshape(IC1, 128, E),
        "g1b": np.asarray(gate1_b, np.float32).reshape(E, 1),
        "g2b": np.asarray(gate2_b, np.float32).reshape(E, 1),
        "bn1g": np.asarray(bn1_gamma, np.float32).reshape(IC1, 128),
        "bn1b": np.asarray(bn1_beta, np.float32).reshape(IC1, 128),
        "obo": (np.asarray(out_b, np.float32) / NCORES).reshape(1, 1),
    }
    g2wf = np.asarray(gate2_W, np.float32)                   # [DHID, E]
    bn2gf = np.asarray(bn2_gamma, np.float32)
    bn2bf = np.asarray(bn2_beta, np.float32)
    owf = np.asarray(out_W, np.float32)                      # [DH2, 1]

    in_maps = []
    for c in range(NCORES):
        js = slice(c * JS1, (c + 1) * JS1)                   # 256-wide slice
        ys = slice(c * 128, (c + 1) * 128)                   # 128-wide out slice
        m = dict(common)
        m["xt"] = np.ascontiguousarray(
            xT[:, c * BL:(c + 1) * BL].reshape(IC1, 128, BL))
        m["w1"] = _round_fp32r(np.ascontiguousarray(
            w1f[:, :, js].reshape(E, IC1, 128, JS1)))
        m["w2"] = np.ascontiguousarray(
            w2f[:, :, ys].reshape(E, DHID // 128, 128, 128)).astype(np.float16)
        m["b2"] = np.ascontiguousarray(b2f[:, ys])
        m["b1"] = np.ascontiguousarray(b1f[:, js].reshape(E, JC1L, 128))
        m["g2w"] = np.ascontiguousarray(g2wf[js, :].reshape(IC2L, 128, E))
        m["bn2g"] = np.ascontiguousarray(bn2gf[js].reshape(JC1L, 128))
        m["bn2b"] = np.ascontiguousarray(bn2bf[js].reshape(JC1L, 128))
        m["ow"] = np.ascontiguousarray(owf[ys].reshape(128, 1))
        in_maps.append(m)

    trace = bool(int(os.environ.get("KERNEL_TRACE", "0")))
    res = run_bass_kernel_spmd(nc, in_maps, list(range(NCORES)), trace=trace)
    kernel._last = res
    y = np.zeros((B, 1), np.float32)
    for c in range(NCORES):
        y += res.results[c]["out"]
    return y


# revision 3
# speedup vs baseline: 1.0040x; 1.0040x over previous
"""MoE network TRN2 kernel: feature-parallel sharding, minimal per-core input bytes.

Collectives are restricted to AllReduce/AllGather only: ReduceScatter in this
runtime triggers a pathological ~100x whole-NEFF slowdown (measured), so
layer 2 is output-feature-sharded and consumes AllGathered activations
instead of combining contraction partials with a ReduceScatter.

Sharding: core c owns output-feature slice j in [256c, 256c+256) of layer-1
experts (exp1_W[:, :, slice], 8 MB fp32r) and the matching input-row slice of
layer-2 experts (exp2_W[:, slice, :], 4 MB fp16). Tokens are sharded only for
the initial BatchNorm stats + gate-1 (512 tokens/core); an AllGather of the
normalized activations gives every core all 4096 tokens, so layer-1 output
z1[:, slice] is complete locally and BN2 statistics are exact with no
cross-core reduce. For layer 2 the normalized activations are re-gathered in
8 token chunks (AllGathers that pipeline ahead of compute) and each core
computes its 128-wide output-feature slice of z2 completely, so only a
partial head contraction leaves the core; partial outputs are summed on the
host. Per-core input bytes: ~14.3 MB (vs 153 MB for replicated weights).
"""
import os
import sys

import numpy as np

sys.path.insert(0, "/opt/trn_rl_repo")

B, DIN, DHID, DH2, E = 4096, 1024, 2048, 1024, 8
NCORES = 8
BL = B // NCORES            # 512 tokens/core for stats+gate1
IC1 = DIN // 128            # 8 input chunks
JS1 = DHID // NCORES        # 256 = layer-1 output slice per core
JC1L = JS1 // 128           # 2 local j chunks
IC2L = JC1L                 # 2 local input chunks for layer 2
JC2 = DH2 // 128            # 8 output chunks, layer 2
NTC = 8                     # token chunks of 512
EPS = 1e-5

_CACHE = {}


def _round_fp32r(x):
    b = np.ascontiguousarray(x, np.float32).view(np.uint32).astype(np.uint64)
    half = np.uint64(1 << 11)
    one = np.uint64(1)
    lsb = (b >> np.uint64(12)) & one
    b = (b + half - one + lsb) & ~np.uint64((1 << 12) - 1)
    return (b & np.uint64(0xFFFFFFFF)).astype(np.uint32).view(np.float32)


def _build():
    import concourse.bass_isa as bass_isa
    import concourse.mybir as mybir
    import concourse.tile as tile
    from concourse import bacc

    f32 = mybir.dt.float32
    f32r = mybir.dt.float32r
    f16 = mybir.dt.float16
    AF = mybir.ActivationFunctionType
    OP = mybir.AluOpType
    RG = [list(range(NCORES))]

    nc = bacc.Bacc(None, target_bir_lowering=False, num_devices=NCORES)
    mask = 0 if os.environ.get("KERNEL_NOCC") else int(os.environ.get("KERNEL_CC_MASK", "15"))
    nocc = mask != 15  # any stubbed collective forces non-Shared buffers
    cc_bn1 = bool(mask & 1)
    cc_ag = bool(mask & 2)
    cc_lg2 = bool(mask & 4)
    cc_rs = bool(mask & 8)

    # ---- per-core inputs (the io-byte budget)
    xt = nc.dram_tensor("xt", [IC1, 128, BL], f32, kind="ExternalInput")
    w1 = nc.dram_tensor("w1", [E, IC1, 128, JS1], f32r, kind="ExternalInput")
    w2 = nc.dram_tensor("w2", [E, DHID // 128, 128, 128], f16,
                        kind="ExternalInput")
    b1 = nc.dram_tensor("b1", [E, JC1L, 128], f32, kind="ExternalInput")
    b2 = nc.dram_tensor("b2", [E, 128], f32, kind="ExternalInput")
    g1w = nc.dram_tensor("g1w", [IC1, 128, E], f32, kind="ExternalInput")
    g2w = nc.dram_tensor("g2w", [IC2L, 128, E], f32, kind="ExternalInput")
    g1b = nc.dram_tensor("g1b", [E, 1], f32, kind="ExternalInput")
    g2b = nc.dram_tensor("g2b", [E, 1], f32, kind="ExternalInput")
    bn1g = nc.dram_tensor("bn1g", [IC1, 128], f32, kind="ExternalInput")
    bn1b = nc.dram_tensor("bn1b", [IC1, 128], f32, kind="ExternalInput")
    bn2g = nc.dram_tensor("bn2g", [JC1L, 128], f32, kind="ExternalInput")
    bn2b = nc.dram_tensor("bn2b", [JC1L, 128], f32, kind="ExternalInput")
    ow = nc.dram_tensor("ow", [128, 1], f32, kind="ExternalInput")
    obo = nc.dram_tensor("obo", [1, 1], f32, kind="ExternalInput")
    out = nc.dram_tensor("out", [B, 1], f32, kind="ExternalOutput")

    with tile.TileContext(nc) as tc:
        with tc.tile_pool(name="const", bufs=1) as const, \
             tc.tile_pool(name="small", bufs=1) as small, \
             tc.tile_pool(name="gsc", bufs=12) as gsc, \
             tc.tile_pool(name="gtmp", bufs=3) as gtmp, \
             tc.tile_pool(name="dram", bufs=1, space="DRAM") as dram:

            # ---- small parameter loads
            bn1g_t = const.tile([128, IC1], f32)
            bn1b_t = const.tile([128, IC1], f32)
            bn2g_t = const.tile([128, JC1L], f32)
            bn2b_t = const.tile([128, JC1L], f32)
            nc.sync.dma_start(out=bn1g_t[:], in_=bn1g.rearrange("c p -> p c"))
            nc.sync.dma_start(out=bn1b_t[:], in_=bn1b.rearrange("c p -> p c"))
            nc.sync.dma_start(out=bn2g_t[:], in_=bn2g.rearrange("c p -> p c"))
            nc.sync.dma_start(out=bn2b_t[:], in_=bn2b.rearrange("c p -> p c"))
            g1w_t = const.tile([128, IC1, E], f32)
            g2w_t = const.tile([128, IC2L, E], f32)
            nc.sync.dma_start(out=g1w_t[:], in_=g1w.rearrange("c p e -> p c e"))
            nc.sync.dma_start(out=g2w_t[:], in_=g2w.rearrange("c p e -> p c e"))
            g1b_t = const.tile([E, 1], f32)
            g2b_t = const.tile([E, 1], f32)
            nc.sync.dma_start(out=g1b_t[:], in_=g1b[:])
            nc.sync.dma_start(out=g2b_t[:], in_=g2b[:])
            b1_tf = small.tile([E, JC1L, 128], f32, name="b1_tf")
            nc.sync.dma_start(out=b1_tf[:], in_=b1[:])
            b1r = const.tile([E, JC1L, 128], f32r)
            nc.vector.tensor_copy(b1r[:], b1_tf[:])
            b2_tf = small.tile([E, 128], f32, name="b2_tf")
            nc.sync.dma_start(out=b2_tf[:], in_=b2[:])
            b2r = const.tile([E, 128], f32r)
            nc.vector.tensor_copy(b2r[:], b2_tf[:])
            ow_t = const.tile([128, 1], f32)
            nc.sync.dma_start(out=ow_t[:], in_=ow[:])
            obo_t = const.tile([128, 1], f32)
            nc.sync.dma_start(
                out=obo_t[:],
                in_=obo[0:1, 0:1].partition_broadcast(128).squeeze(1))

            junk = const.tile([128, 512], f32)

            # ---- helpers
            def bn_finish(s1, s2, icn, gamma_t, beta_t, name):
                mu = small.tile([128, icn], f32, name=f"mu_{name}")
                ex2 = small.tile([128, icn], f32, name=f"ex2_{name}")
                nc.vector.tensor_scalar(mu[:], s1[:], 1.0 / B, None, OP.mult)
                nc.vector.tensor_scalar(ex2[:], s2[:], 1.0 / B, None, OP.mult)
                var = small.tile([128, icn], f32, name=f"var_{name}")
                nc.vector.tensor_tensor(out=var[:], in0=mu[:], in1=mu[:], op=OP.mult)
                nc.vector.tensor_tensor(out=var[:], in0=ex2[:], in1=var[:],
                                        op=OP.subtract)
                vare = small.tile([128, icn], f32, name=f"vare_{name}")
                nc.vector.tensor_scalar(vare[:], var[:], EPS, None, OP.add)
                sd = small.tile([128, icn], f32, name=f"sd_{name}")
                nc.scalar.activation(sd[:], vare[:], AF.Sqrt)
                rstd = small.tile([128, icn], f32, name=f"rstd_{name}")
                nc.vector.reciprocal(rstd[:], sd[:])
                sv = small.tile([128, icn], f32, name=f"sv_{name}")
                bv = small.tile([128, icn], f32, name=f"bv_{name}")
                nc.vector.tensor_tensor(out=sv[:], in0=rstd[:], in1=gamma_t[:],
                                        op=OP.mult)
                nc.vector.tensor_tensor(out=bv[:], in0=mu[:], in1=sv[:], op=OP.mult)
                nc.vector.tensor_tensor(out=bv[:], in0=beta_t[:], in1=bv[:],
                                        op=OP.subtract)
                return sv, bv

            def gating_chain(lg, width, name):
                """top-2 masked softmax over E=8 partitions; lg [E, width] f32.
                Returns gat tile [E, width] f32 from the gtmp pool."""
                m1 = gsc.tile([E, width], f32, tag="g", name=f"m1_{name}")
                nc.gpsimd.partition_all_reduce(m1[:], lg[:], channels=E,
                                               reduce_op=bass_isa.ReduceOp.max)
                ismax = gsc.tile([E, width], f32, tag="g", name=f"ismax_{name}")
                nc.vector.tensor_tensor(out=ismax[:], in0=lg[:], in1=m1[:],
                                        op=OP.is_equal)
                cnt = gsc.tile([E, width], f32, tag="g", name=f"cnt_{name}")
                nc.gpsimd.partition_all_reduce(cnt[:], ismax[:], channels=E,
                                               reduce_op=bass_isa.ReduceOp.add)
                tmp = gsc.tile([E, width], f32, tag="g", name=f"tmp_{name}")
                nc.vector.scalar_tensor_tensor(
                    out=tmp[:], in0=ismax[:], scalar=-1e30, in1=lg[:],
                    op0=OP.mult, op1=OP.add)
                m2 = gsc.tile([E, width], f32, tag="g", name=f"m2_{name}")
                nc.gpsimd.partition_all_reduce(m2[:], tmp[:], channels=E,
                                               reduce_op=bass_isa.ReduceOp.max)
                c2m = gsc.tile([E, width], f32, tag="g", name=f"c2m_{name}")
                nc.vector.tensor_scalar(c2m[:], cnt[:], 1.5, None, OP.is_ge)
                dif = gsc.tile([E, width], f32, tag="g", name=f"dif_{name}")
                nc.vector.tensor_tensor(out=dif[:], in0=m1[:], in1=m2[:],
                                        op=OP.subtract)
                nc.vector.tensor_tensor(out=dif[:], in0=dif[:], in1=c2m[:],
                                        op=OP.mult)
                v2 = gsc.tile([E, width], f32, tag="g", name=f"v2_{name}")
                nc.vector.tensor_tensor(out=v2[:], in0=dif[:], in1=m2[:], op=OP.add)
                msk = gsc.tile([E, width], f32, tag="g", name=f"msk_{name}")
                nc.vector.tensor_tensor(out=msk[:], in0=lg[:], in1=v2[:],
                                        op=OP.is_ge)
                d = gsc.tile([E, width], f32, tag="g", name=f"d_{name}")
                nc.vector.tensor_tensor(out=d[:], in0=lg[:], in1=m1[:],
                                        op=OP.subtract)
                exd = gsc.tile([E, width], f32, tag="g", name=f"exd_{name}")
                nc.scalar.activation(exd[:], d[:], AF.Exp)
                exm = gsc.tile([E, width], f32, tag="g", name=f"exm_{name}")
                nc.vector.tensor_tensor(out=exm[:], in0=exd[:], in1=msk[:],
                                        op=OP.mult)
                den = gsc.tile([E, width], f32, tag="g", name=f"den_{name}")
                nc.gpsimd.partition_all_reduce(den[:], exm[:], channels=E,
                                               reduce_op=bass_isa.ReduceOp.add)
                rden = gsc.tile([E, width], f32, tag="g", name=f"rden_{name}")
                nc.vector.reciprocal(rden[:], den[:])
                gat = gtmp.tile([E, width], f32, tag="gat", name=f"gat_{name}")
                nc.vector.tensor_tensor(out=gat[:], in0=exm[:], in1=rden[:],
                                        op=OP.mult)
                return gat

            # gates for all tokens as f32r (bias-matmul rhs); persistent
            g1ar = small.tile([E, B], f32r, name="g1ar")
            g2ar = small.tile([E, B], f32r, name="g2ar")
            ysb = small.tile([128, 4 * NTC], f32, name="ysb")

            # collective DRAM buffers
            bn1_pl = dram.tile([128, 2 * IC1], f32, name="bn1_pl")
            _shr = "Local"
            bn1_ps = dram.tile([128, 2 * IC1], f32, addr_space=_shr,
                               name="bn1_ps")
            ag_xi = dram.tile([128, IC1, BL], f32, name="ag_xi")
            ag_xo = dram.tile([NCORES, 128, IC1, BL], f32, addr_space=_shr,
                              name="ag_xo")
            ag_gi = dram.tile([E, BL], f32, name="ag_gi")
            ag_go = dram.tile([NCORES, E, BL], f32, addr_space=_shr,
                              name="ag_go")
            lg2_pl = dram.tile([E, B], f32, name="lg2_pl")
            lg2_ps = dram.tile([E, B], f32, name="lg2_ps")
            gd2 = dram.tile([E, B], f16, name="gd2")
            ag2_is = [dram.tile([128, IC2L, 512], f16, name=f"ag2i_{t8}")
                      for t8 in range(NTC)]
            ag2_os = [dram.tile([NCORES, 128, IC2L, 512], f16,
                                name=f"ag2o_{t8}") for t8 in range(NTC)]

            # ================= phase 0: BN1 stats (AllReduce) + gate1 =========
            with tc.tile_pool(name="p0", bufs=1) as p0:
                xtf = p0.tile([128, IC1, BL], f32, name="xtf")
                nc.sync.dma_start(out=xtf[:], in_=xt.rearrange("c p t -> p c t"))
                s1x = small.tile([128, IC1], f32, name="s1x")
                s2x = small.tile([128, IC1], f32, name="s2x")
                for ic in range(IC1):
                    nc.vector.tensor_reduce(s1x[:, ic:ic + 1], xtf[:, ic, :],
                                            mybir.AxisListType.X, OP.add)
                    nc.scalar.activation(junk[:], xtf[:, ic, :], AF.Square,
                                         accum_out=s2x[:, ic:ic + 1])
                pk = small.tile([128, 2 * IC1], f32, name="pk_bn1")
                nc.vector.tensor_copy(pk[:, :IC1], s1x[:])
                nc.vector.tensor_copy(pk[:, IC1:], s2x[:])
                nc.sync.dma_start(out=bn1_pl[:], in_=pk[:])
                if not cc_bn1:
                    nc.sync.dma_start(out=bn1_ps[:], in_=bn1_pl[:])
                else:
                    nc.gpsimd.collective_compute(
                        "AllReduce", OP.add, replica_groups=RG,
                        ins=[bn1_pl[:]], outs=[bn1_ps[:]])
                gl = small.tile([128, 2 * IC1], f32, name="gl_bn1")
                nc.sync.dma_start(out=gl[:], in_=bn1_ps[:])
                sv1, bv1 = bn_finish(gl[:, :IC1], gl[:, IC1:], IC1,
                                     bn1g_t, bn1b_t, "bn1")

                xnc = p0.tile([128, IC1, BL], f32, name="xnc")
                for ic in range(IC1):
                    nc.vector.tensor_scalar(xnc[:, ic, :], xtf[:, ic, :],
                                            sv1[:, ic:ic + 1], bv1[:, ic:ic + 1],
                                            OP.mult, OP.add)

                # gate1 on local tokens (exact fp32)
                with tc.tile_pool(name="psg1", bufs=1, space="PSUM") as psg1:
                    psg = psg1.tile([E, BL], f32)
                    for ic in range(IC1):
                        nc.tensor.matmul(psg[:], lhsT=g1w_t[:, ic, :],
                                         rhs=xnc[:, ic, :],
                                         start=(ic == 0), stop=(ic == IC1 - 1))
                    lg1 = gsc.tile([E, BL], f32, tag="g", name="lg1")
                    nc.vector.tensor_scalar(lg1[:], psg[:], g1b_t[:], None, OP.add)
                gat1 = gating_chain(lg1, BL, "g1")

                # AllGather normalized activations + gates
                nc.sync.dma_start(out=ag_xi[:], in_=xnc[:])
                if not cc_ag:
                    for _r in range(NCORES):
                        nc.sync.dma_start(out=ag_xo[_r], in_=ag_xi[:])
                else:
                    nc.gpsimd.collective_compute(
                        "AllGather", OP.bypass, replica_groups=RG,
                        ins=[ag_xi[:]], outs=[ag_xo[:]])
                nc.sync.dma_start(out=ag_gi[:], in_=gat1[:])
                if not cc_ag:
                    for _r in range(NCORES):
                        nc.sync.dma_start(out=ag_go[_r], in_=ag_gi[:])
                else:
                    nc.gpsimd.collective_compute(
                        "AllGather", OP.bypass, replica_groups=RG,
                        ins=[ag_gi[:]], outs=[ag_go[:]])

            # unpack gates for all tokens -> f32r
            for r in range(NCORES):
                gtr = gtmp.tile([E, BL], f32, tag="gat", name=f"g1u_{r}")
                nc.sync.dma_start(out=gtr[:], in_=ag_go[r])
                nc.vector.tensor_copy(g1ar[:, r * BL:(r + 1) * BL], gtr[:])

            s1z = small.tile([128, JC1L, NTC], f32, name="s1z")
            s2z = small.tile([128, JC1L, NTC], f32, name="s2z")

            with tc.tile_pool(name="xn2p", bufs=1) as xn2p:
                xn2 = xn2p.tile([128, IC2L, B], f32)

                # ============= layer 1: feature-sliced, all tokens ============
                with tc.tile_pool(name="z1p", bufs=1) as z1p:
                    z1sb = z1p.tile([128, JC1L, B], f32)
                    with tc.tile_pool(name="xqp", bufs=2) as xqp, \
                         tc.tile_pool(name="htp", bufs=5) as htp, \
                         tc.tile_pool(name="gb1p", bufs=4) as gb1p, \
                         tc.tile_pool(name="w1p", bufs=10) as w1p, \
                         tc.tile_pool(name="ps1", bufs=8, space="PSUM") as ps1:
                        for t8 in range(NTC):
                            sl = slice(t8 * 512, (t8 + 1) * 512)
                            xq = xqp.tile([128, IC1, 512], f32, tag="xq",
                                          name=f"xq_{t8}")
                            nc.sync.dma_start(out=xq[:], in_=ag_xo[t8])
                            psq = [ps1.tile([128, 512], f32, tag="ps",
                                            name=f"ps1_{t8}_{jc}")
                                   for jc in range(JC1L)]
                            for e in range(E):
                                gb = gb1p.tile([128, 512], f32, tag="gb",
                                               name=f"gb_{t8}_{e}")
                                nc.sync.dma_start(
                                    out=gb[:],
                                    in_=ag_go[t8, e:e + 1, :]
                                    .partition_broadcast(128).squeeze(1))
                                for ic in range(IC1):
                                    w1t = w1p.tile([128, JS1], f32r, tag="ws",
                                                   name=f"w1_{t8}_{e}_{ic}")
                                    nc.sync.dma_start(out=w1t[:], in_=w1[e, ic])
                                    ht = htp.tile([128, 512], f32r, tag="ht",
                                                  name=f"ht_{t8}_{e}_{ic}")
                                    nc.vector.tensor_tensor(
                                        out=ht[:], in0=xq[:, ic, :], in1=gb[:],
                                        op=OP.mult)
                                    for jc in range(JC1L):
                                        nc.tensor.matmul(
                                            psq[jc][:],
                                            lhsT=w1t[:, jc * 128:(jc + 1) * 128],
                                            rhs=ht[:],
                                            start=(e == 0 and ic == 0),
                                            stop=False)
                            for jc in range(JC1L):
                                nc.tensor.matmul(
                                    psq[jc][:], lhsT=b1r[:, jc, :],
                                    rhs=g1ar[:, sl], start=False, stop=True)
                                nc.vector.tensor_copy(z1sb[:, jc, sl], psq[jc][:])
                                nc.vector.tensor_reduce(
                                    s1z[:, jc, t8:t8 + 1], z1sb[:, jc, sl],
                                    mybir.AxisListType.X, OP.add)
                                nc.scalar.activation(
                                    junk[:], z1sb[:, jc, sl], AF.Square,
                                    accum_out=s2z[:, jc, t8:t8 + 1])

                    # ============= BN2 (local, exact) + relu ==================
                    s1f = small.tile([128, JC1L], f32, name="s1f")
                    s2f = small.tile([128, JC1L], f32, name="s2f")
                    for jc in range(JC1L):
                        nc.vector.tensor_reduce(s1f[:, jc:jc + 1], s1z[:, jc, :],
                                                mybir.AxisListType.X, OP.add)
                        nc.vector.tensor_reduce(s2f[:, jc:jc + 1], s2z[:, jc, :],
                                                mybir.AxisListType.X, OP.add)
                    sv2, bv2 = bn_finish(s1f, s2f, JC1L, bn2g_t, bn2b_t, "bn2")
                    for jc in range(JC1L):
                        nc.scalar.activation(xn2[:, jc, :], z1sb[:, jc, :],
                                             AF.Relu, bias=bv2[:, jc:jc + 1],
                                             scale=sv2[:, jc:jc + 1])

                # ==== gate2 + xn2 re-gather ===================================
                # cc-stream order matters (collectives execute serially on the
                # gpsimd queue): one 128 KB AllReduce for all gate-2 logits,
                # then the top-2 chains, then the 8 chunked fp16 AllGathers —
                # so layer 2's first chunk is ready ~50 us after BN2.
                with tc.tile_pool(name="psg2", bufs=2, space="PSUM") as psg2:
                    for t8 in range(NTC):
                        sl = slice(t8 * 512, (t8 + 1) * 512)
                        xf = gtmp.tile([128, IC2L, 512], f16, tag="xf",
                                       name=f"xf_{t8}", bufs=2)
                        nc.vector.tensor_copy(xf[:], xn2[:, :, sl])
                        nc.sync.dma_start(out=ag2_is[t8][:], in_=xf[:])
                        psl = psg2.tile([E, 512], f32, tag="ps",
                                        name=f"pg2_{t8}")
                        for ic in range(IC2L):
                            nc.tensor.matmul(psl[:], lhsT=g2w_t[:, ic, :],
                                             rhs=xn2[:, ic, sl],
                                             start=(ic == 0),
                                             stop=(ic == IC2L - 1))
                        lgs = gtmp.tile([E, 512], f32, tag="gat",
                                        name=f"lgp_{t8}")
                        nc.vector.tensor_copy(lgs[:], psl[:])
                        nc.sync.dma_start(out=lg2_pl[:, sl], in_=lgs[:])
                if not cc_lg2:
                    nc.sync.dma_start(out=lg2_ps[:], in_=lg2_pl[:])
                else:
                    nc.gpsimd.collective_compute(
                        "AllReduce", OP.add, replica_groups=RG,
                        ins=[lg2_pl[:]], outs=[lg2_ps[:]])
                for t8 in range(NTC):
                    sl = slice(t8 * 512, (t8 + 1) * 512)
                    lgc = gsc.tile([E, 512], f32, tag="g", name=f"lg2_{t8}")
                    nc.sync.dma_start(out=lgc[:], in_=lg2_ps[:, sl])
                    nc.vector.tensor_scalar(lgc[:], lgc[:], g2b_t[:], None,
                                            OP.add)
                    gat2 = gating_chain(lgc, 512, f"g2_{t8}")
                    nc.vector.tensor_copy(g2ar[:, sl], gat2[:])
                    g16 = gtmp.tile([E, 512], f16, tag="gat",
                                    name=f"g16_{t8}")
                    nc.vector.tensor_copy(g16[:], gat2[:])
                    nc.sync.dma_start(out=gd2[:, sl], in_=g16[:])
                for t8 in range(NTC):
                    if not cc_rs:
                        for _r in range(NCORES):
                            nc.sync.dma_start(out=ag2_os[t8][_r],
                                              in_=ag2_is[t8][:])
                    else:
                        nc.gpsimd.collective_compute(
                            "AllGather", OP.bypass, replica_groups=RG,
                            ins=[ag2_is[t8][:]], outs=[ag2_os[t8][:]])

                with tc.tile_pool(name="w2p", bufs=1) as w2p, \
                     tc.tile_pool(name="xt2p", bufs=3) as xt2p, \
                     tc.tile_pool(name="ht2p", bufs=3) as ht2p, \
                     tc.tile_pool(name="gb2p", bufs=10) as gb2p, \
                     tc.tile_pool(name="z2p", bufs=2) as z2p, \
                     tc.tile_pool(name="ps2", bufs=4, space="PSUM") as ps2, \
                     tc.tile_pool(name="psh", bufs=2, space="PSUM") as psh:
                    w2sb = w2p.tile([128, E, DHID // 128, 128], f16)
                    nc.sync.dma_start(out=w2sb[:],
                                      in_=w2.rearrange("e i p f -> p e i f"))
                    for t8 in range(NTC):
                        sl = slice(t8 * 512, (t8 + 1) * 512)
                        gb2s = []
                        for e in range(E):
                            gb2 = gb2p.tile([128, 512], f16, tag="gb2",
                                            name=f"gb2_{t8}_{e}")
                            nc.sync.dma_start(
                                out=gb2[:],
                                in_=gd2[e:e + 1, sl].partition_broadcast(128)
                                .squeeze(1))
                            gb2s.append(gb2)
                        p2 = ps2.tile([128, 512], f32, tag="ps",
                                      name=f"p2_{t8}")
                        for r in range(NCORES):
                            for ic in range(IC2L):
                                xt2 = xt2p.tile([128, 512], f16, tag="xt2",
                                                name=f"xt2_{t8}_{r}_{ic}")
                                nc.sync.dma_start(out=xt2[:],
                                                  in_=ag2_os[t8][r, :, ic, :])
                                for e in range(E):
                                    ht2 = ht2p.tile([128, 512], f16, tag="ht2",
                                                    name=f"ht2_{t8}_{r}_{ic}_{e}")
                                    nc.vector.tensor_tensor(
                                        out=ht2[:], in0=xt2[:], in1=gb2s[e][:],
                                        op=OP.mult)
                                    nc.tensor.matmul(
                                        p2[:],
                                        lhsT=w2sb[:, e, r * IC2L + ic, :],
                                        rhs=ht2[:],
                                        start=(r == 0 and ic == 0 and e == 0),
                                        stop=False)
                        nc.tensor.matmul(p2[:], lhsT=b2r[:], rhs=g2ar[:, sl],
                                         start=False, stop=True)
                        z2a = z2p.tile([128, 512], f32, tag="z2",
                                       name=f"z2a_{t8}")
                        nc.scalar.activation(z2a[:], p2[:], AF.Relu)
                        for k in range(4):
                            ph = psh.tile([128, 1], f32, tag="ph",
                                          name=f"ph_{t8}_{k}")
                            nc.tensor.matmul(ph[:],
                                             lhsT=z2a[:, k * 128:(k + 1) * 128],
                                             rhs=ow_t[:], start=True, stop=True)
                            nc.vector.tensor_scalar(
                                ysb[:, t8 * 4 + k:t8 * 4 + k + 1],
                                ph[:], obo_t[:], None, OP.add)

            nc.sync.dma_start(out=out.rearrange("(c p) m -> p (c m)", p=128),
                              in_=ysb[:])

    nc.finalize()
    return nc


def _get_nc():
    if "nc" not in _CACHE:
        _CACHE["nc"] = _build()
    return _CACHE["nc"]


def kernel(x, bn1_gamma, bn1_beta, bn2_gamma, bn2_beta,
           gate1_W, gate1_b, exp1_W, exp1_b,
           gate2_W, gate2_b, exp2_W, exp2_b,
           out_W, out_b):
    from concourse.bass_utils import run_bass_kernel_spmd

    nc = _get_nc()

    xT = np.ascontiguousarray(np.asarray(x, np.float32).T)   # [DIN, B]
    w1f = np.asarray(exp1_W, np.float32)                     # [E, DIN, DHID]
    w2f = np.asarray(exp2_W, np.float32)                     # [E, DHID, DH2]
    b1f = np.asarray(exp1_b, np.float32)                     # [E, DHID]
    b2f = np.asarray(exp2_b, np.float32)                     # [E, DH2]
    common = {
        "g1w": np.asarray(gate1_W, np.float32).reshape(IC1, 128, E),
        "g1b": np.asarray(gate1_b, np.float32).reshape(E, 1),
        "g2b": np.asarray(gate2_b, np.float32).reshape(E, 1),
        "bn1g": np.asarray(bn1_gamma, np.float32).reshape(IC1, 128),
        "bn1b": np.asarray(bn1_beta, np.float32).reshape(IC1, 128),
        "obo": (np.asarray(out_b, np.float32) / NCORES).reshape(1, 1),
    }
    g2wf = np.asarray(gate2_W, np.float32)                   # [DHID, E]
    bn2gf = np.asarray(bn2_gamma, np.float32)
    bn2bf = np.asarray(bn2_beta, np.float32)
    owf = np.asarray(out_W, np.float32)                      # [DH2, 1]

    in_maps = []
    for c in range(NCORES):
        js = slice(c * JS1, (c + 1) * JS1)                   # 256-wide slice
        ys = slice(c * 128, (c + 1) * 128)                   # 128-wide out slice
        m = dict(common)
        m["xt"] = np.ascontiguousarray(
            xT[:, c * BL:(c + 1) * BL].reshape(IC1, 128, BL))
        m["w1"] = _round_fp32r(np.ascontiguousarray(
            w1f[:, :, js].reshape(E, IC1, 128, JS1)))
        m["w2"] = np.ascontiguousarray(
            w2f[:, :, ys].reshape(E, DHID // 128, 128, 128)).astype(np.float16)
        m["b2"] = np.ascontiguousarray(b2f[:, ys])
        m["b1"] = np.ascontiguousarray(b1f[:, js].reshape(E, JC1L, 128))
        m["g2w"] = np.ascontiguousarray(g2wf[js, :].reshape(IC2L, 128, E))
        m["bn2g"] = np.ascontiguousarray(bn2gf[js].reshape(JC1L, 128))
        m["bn2b"] = np.ascontiguousarray(bn2bf[js].reshape(JC1L, 128))
        m["ow"] = np.ascontiguousarray(owf[ys].reshape(128, 1))
        in_maps.append(m)

    trace = bool(int(os.environ.get("KERNEL_TRACE", "0")))
    res = run_bass_kernel_spmd(nc, in_maps, list(range(NCORES)), trace=trace)
    kernel._last = res
    y = np.zeros((B, 1), np.float32)
    for c in range(NCORES):
        y += res.results[c]["out"]
    return y
